# revision 32
# baseline (speedup 1.0000x reference)
"""Trainium2 Bass kernel for multi-head attention (b=2, n=2048, d=512, h=8).

Sharding: batch*heads across 8 cores — each core computes full-sequence
attention for 2 heads of one batch element, plus its partial contribution
to the output projection. Host sums the 4 per-core partials per batch.

Per-core dataflow (fp16 operands, fp32 PSUM accumulation):
  A) Q.T, K.T = W.T-chunked @ x.T  (dh2=128 on partitions);
     V.T likewise, PE-transposed to V (keys on partitions) + fused ones col
  B) flash-style attention per 512-query tile, software-pipelined one tile
     deep: while ScalarE streams exp for tile t (both heads packed in one
     ACTIVATE; scale folded in), the PE runs PV+rowsum for tile t-1
     ([V_h|1].T @ P.T, accumulated in PSUM), normalization and the output
     projection for t-1 trail behind. Head0 runs on PE rows/cols 0-63 and
     head1 on 64-127 wherever both appear (row-tiled concurrent matmuls).
  C) partial out = O_norm.T.T @ Wo-cols (both heads as a concurrent
     K-split pair), DVE-copied to SBUF, DMA out fp16.
"""

import numpy as np

import concourse.bass as bass
import concourse.mybir as mybir
import concourse.tile as tile
from concourse import bacc
from concourse.bass_utils import run_bass_kernel_spmd
from concourse.masks import make_identity
from contextlib import ExitStack

P = 128          # partitions / key-chunk size
N = 2048         # sequence length
D = 512          # model dim
DH2 = 128        # head_dim * 2 local heads
QT = 512         # queries per tile
NQT = N // QT    # 4 query tiles
KC = N // P      # 16 key chunks
SCALE = D ** -0.5
F32 = mybir.dt.float32
F16 = mybir.dt.float16
EXP = mybir.ActivationFunctionType.Exp

_CACHED = {}


def build_nc():
    nc = bacc.Bacc("TRN2", target_bir_lowering=False, debug=False, num_devices=8)

    xt_d = nc.dram_tensor("xt", [P, 4, N], F16, kind="ExternalInput")
    wq_d = nc.dram_tensor("wq", [P, 4, DH2], F16, kind="ExternalInput")
    wk_d = nc.dram_tensor("wk", [P, 4, DH2], F16, kind="ExternalInput")
    wv_d = nc.dram_tensor("wv", [P, 4, DH2], F16, kind="ExternalInput")
    wo_d = nc.dram_tensor("wo", [64, 2, D], F16, kind="ExternalInput")
    out_d = nc.dram_tensor("out", [N, D], F16, kind="ExternalOutput")

    with tile.TileContext(nc) as tc, ExitStack() as ctx:
        const = ctx.enter_context(tc.tile_pool(name="const", bufs=1))
        xt_pool = ctx.enter_context(tc.tile_pool(name="xt", bufs=1))
        w_pool = ctx.enter_context(tc.tile_pool(name="w", bufs=1))
        qk_pool = ctx.enter_context(tc.tile_pool(name="qk", bufs=1))
        v_pool = ctx.enter_context(tc.tile_pool(name="v", bufs=1))
        pt_pool = ctx.enter_context(tc.tile_pool(name="pt", bufs=20))
        on_pool = ctx.enter_context(tc.tile_pool(name="on", bufs=2))
        nrm_pool = ctx.enter_context(tc.tile_pool(name="nrm", bufs=2))
        y_sb_pool = ctx.enter_context(tc.tile_pool(name="ysb", bufs=2))
        st_pool = ctx.enter_context(tc.tile_pool(name="st_ps", bufs=2, space="PSUM"))
        ot_pool = ctx.enter_context(tc.tile_pool(name="ot_ps", bufs=1, space="PSUM"))
        misc_ps = ctx.enter_context(tc.tile_pool(name="misc_ps", bufs=2, space="PSUM"))

        identity = const.tile([P, P], F16)
        make_identity(nc, identity[:])

        def fill_ones(out_ap):
            # DVE (in*0)+1 — writes exact fp16 ones
            nc.vector.tensor_scalar(
                out=out_ap, in0=identity[:, 0:out_ap.free_size()],
                scalar1=0.0, scalar2=1.0,
                op0=mybir.AluOpType.mult, op1=mybir.AluOpType.add,
            )

        # ---- input DMAs (fp16): x as 16 column-slice DMAs spread across
        # both HWDGE rings (sync + scalar) so many SDMA engines pull in
        # parallel; first slices match the first projection matmuls ----
        wk_sb = w_pool.tile([P, 4, DH2], F16, tag="wk")
        nc.sync.dma_start(wk_sb[:], wk_d.ap())
        wq_sb = w_pool.tile([P, 4, DH2], F16, tag="wq")
        nc.scalar.dma_start(wq_sb[:], wq_d.ap())
        wv_sb = w_pool.tile([P, 4, DH2], F16, tag="wv")
        nc.sync.dma_start(wv_sb[:], wv_d.ap())
        wo_sb = w_pool.tile([64, 2, D], F16, tag="wo")
        nc.scalar.dma_start(wo_sb[:], wo_d.ap())
        xt_sb = xt_pool.tile([P, 4, N], F16, tag="xt")
        rings = [nc.sync, nc.scalar]
        i = 0
        for tq4 in range(NQT):
            qs = slice(tq4 * QT, (tq4 + 1) * QT)
            for c in range(4):
                rings[i % 2].dma_start(xt_sb[:, c, qs], xt_d.ap()[:, c, qs])
                i += 1

        # ---- stage A: K and Q projections (V happens inside the t=0 slot) ----
        qT = qk_pool.tile([P, N], F16, tag="qT")
        kT = qk_pool.tile([P, N], F16, tag="kT")
        vT = qk_pool.tile([P, N], F16, tag="vT")
        v_sb = v_pool.tile([P, KC, 130], F16)

        def proj_tile(tgt, w_sb, tq4):
            ps = misc_ps.tile([P, QT], F32, tag="m", name=f"ps_{tgt.name}_{tq4}")
            for c in range(4):
                nc.tensor.matmul(
                    ps[:],
                    lhsT=w_sb[:, c, :],
                    rhs=xt_sb[:, c, tq4 * QT:(tq4 + 1) * QT],
                    start=(c == 0), stop=(c == 3),
                )
            nc.vector.tensor_copy(tgt[:, tq4 * QT:(tq4 + 1) * QT], ps[:])

        def v_path():
            for tq4 in range(NQT):
                ps = misc_ps.tile([P, QT], F32, tag="m", name=f"ps_v_{tq4}")
                for c in range(4):
                    nc.tensor.matmul(
                        ps[:],
                        lhsT=wv_sb[:, c, :],
                        rhs=xt_sb[:, c, tq4 * QT:(tq4 + 1) * QT],
                        start=(c == 0), stop=(c == 3),
                    )
                nc.vector.tensor_copy(vT[:, tq4 * QT:(tq4 + 1) * QT], ps[:])
            fill_ones(v_sb[:, :, 64])
            fill_ones(v_sb[:, :, 129])
            for c in range(KC):
                tp = misc_ps.tile([P, P], F16, tag="m", name=f"tp_{c}")
                nc.tensor.transpose(tp[:], vT[:, c * P:(c + 1) * P], identity[:])
                nc.vector.tensor_copy(v_sb[:, c, 0:64], tp[:, 0:64])
                nc.vector.tensor_copy(v_sb[:, c, 65:129], tp[:, 64:128])

        # ---- stage B/C: pipelined over query tiles ----
        pts = {}   # (t, c) -> pt tile
        ots = {}   # t -> (ot0, ot1)

        def st_act(t):
            st_act0(t, 0, KC)

        def st_act0(t, c0, c1):
            tq = slice(t * QT, (t + 1) * QT)
            for c in range(c0, c1):
                st = st_pool.tile([P, 2 * QT], F32, tag="st", name=f"st_{t}_{c}")
                for h in range(2):
                    hp = 64 * h
                    nc.tensor.matmul(
                        st[:, h * QT:(h + 1) * QT],
                        lhsT=kT[hp:hp + 64, c * P:(c + 1) * P],
                        rhs=qT[hp:hp + 64, tq],
                        start=True, stop=True,
                    )
                pt = pt_pool.tile([P, 2 * QT], F16, tag="pt", name=f"pt_{t}_{c}")
                nc.scalar.activation(pt[:], st[:], EXP, scale=SCALE)
                pts[(t, c)] = pt

        def pv_chunks(t, c0, c1):
            if c0 == 0:
                ots[t] = (ot_pool.tile([65, QT], F32, tag="ot0", name=f"ot0_{t}"),
                          ot_pool.tile([65, QT], F32, tag="ot1", name=f"ot1_{t}"))
            for c in range(c0, c1):
                pt = pts.pop((t, c))
                for h in range(2):
                    nc.tensor.matmul(
                        ots[t][h][:],
                        lhsT=v_sb[:, c, 65 * h:65 * h + 65],
                        rhs=pt[:, h * QT:(h + 1) * QT],
                        start=(c == 0), stop=(c == KC - 1),
                        skip_group_check=True,
                    )

        on_ts = {}

        def norm_head(t, h):
            # normalize one head (all operands partition-aligned at base 0)
            ot = ots[t][h]
            sums = nrm_pool.tile([1, QT], F32, tag="sums", name=f"sums_{t}_{h}")
            nc.vector.tensor_copy(sums[:], ot[64:65, :])
            rsum = nrm_pool.tile([1, QT], F32, tag="rsum", name=f"rsum_{t}_{h}")
            nc.vector.reciprocal_approx_fast(rsum[:], sums[:])
            rcb = nrm_pool.tile([64, QT], F32, tag="rcb", name=f"rcb_{t}_{h}")
            nc.gpsimd.partition_broadcast(rcb[:], rsum[:], channels=64)
            on_h = on_pool.tile([64, QT], F16, tag=f"on{h}", name=f"on_{t}_{h}")
            nc.vector.tensor_mul(on_h[:], ot[0:64, :], rcb[:])
            on_ts.setdefault(t, []).append(on_h)

        def out_proj(t):
            on_t = on_ts.pop(t)
            del ots[t]
            for qc in range(4):
                yps = misc_ps.tile([P, D], F32, tag="m", name=f"y_{t}_{qc}")
                for h in range(2):
                    nc.tensor.matmul(
                        yps[:],
                        lhsT=on_t[h][:, qc * P:(qc + 1) * P],
                        rhs=wo_sb[:, h, :],
                        start=(h == 0), stop=(h == 1),
                    )
                ysb = y_sb_pool.tile([P, D], F16, tag="ysb", name=f"ysb_{t}_{qc}")
                nc.vector.tensor_copy(ysb[:], yps[:])
                nc.sync.dma_start(
                    out_d.ap()[(t * 4 + qc) * P:(t * 4 + qc + 1) * P, :], ysb[:])

        # t=0: interleave K-projections with the first ST/ACT groups so the
        # exp stream starts as soon as kT-tile0 + qT-tile0 exist
        proj_tile(kT, wk_sb, 0)
        proj_tile(qT, wq_sb, 0)
        for tq4 in range(NQT):
            if tq4 > 0:
                proj_tile(kT, wk_sb, tq4)
            st_act0(0, tq4 * 4, tq4 * 4 + 4)
            if tq4 == 1:
                for q4 in range(1, NQT):
                    proj_tile(qT, wq_sb, q4)
            if tq4 == 2:
                v_path()

        # PV for tile t-1 interleaves with the ST/ACT stream of tile t in
        # 4-chunk blocks so the exp stream never waits behind a PV burst
        for t in range(1, NQT + 1):
            for g in range(4):
                if t < NQT:
                    st_act0(t, 4 * g, 4 * g + 4)
                pv_chunks(t - 1, 4 * g, 4 * g + 4)
            norm_head(t - 1, 0)
            norm_head(t - 1, 1)
            out_proj(t - 1)

    nc.compile()
    return nc


def make_in_maps(x, Wq, Wk, Wv, Wo):
    """Shard full inputs into the 8 per-core input dicts (host-side fp16)."""
    in_maps = []
    for core in range(8):
        b, p = divmod(core, 4)
        r = slice(p * DH2, (p + 1) * DH2)
        # xt[p, c, n] = x[b, n, c*128 + p]
        xt = x[b].T.reshape(4, P, N).transpose(1, 0, 2)
        wq = Wq[r, :].T.reshape(4, P, DH2).transpose(1, 0, 2)
        wk = Wk[r, :].T.reshape(4, P, DH2).transpose(1, 0, 2)
        wv = Wv[r, :].T.reshape(4, P, DH2).transpose(1, 0, 2)
        wo = Wo[:, r].T.reshape(2, 64, D).transpose(1, 0, 2)
        in_maps.append({
            "xt": np.ascontiguousarray(xt, dtype=np.float16),
            "wq": np.ascontiguousarray(wq, dtype=np.float16),
            "wk": np.ascontiguousarray(wk, dtype=np.float16),
            "wv": np.ascontiguousarray(wv, dtype=np.float16),
            "wo": np.ascontiguousarray(wo, dtype=np.float16),
        })
    return in_maps


def kernel(x, mask, Wq, Wk, Wv, Wo, bo, _trace=False):
    x = np.asarray(x, dtype=np.float32)
    Wq = np.asarray(Wq, dtype=np.float32)
    Wk = np.asarray(Wk, dtype=np.float32)
    Wv = np.asarray(Wv, dtype=np.float32)
    Wo = np.asarray(Wo, dtype=np.float32)
    bo = np.asarray(bo, dtype=np.float32)
    # mask is additive and all-zeros per the problem spec -> identity, ignored

    if "nc" not in _CACHED:
        _CACHED["nc"] = build_nc()
    nc = _CACHED["nc"]

    in_maps = make_in_maps(x, Wq, Wk, Wv, Wo)
    res = run_bass_kernel_spmd(nc, in_maps, core_ids=list(range(8)), trace=_trace)
    parts = [res.results[c]["out"].astype(np.float32) for c in range(8)]
    out = np.empty((2, N, D), dtype=np.float32)
    for b in range(2):
        out[b] = parts[4 * b] + parts[4 * b + 1] + parts[4 * b + 2] + parts[4 * b + 3]
    out += bo[None, None, :]
    _CACHED["last_exec_time_ns"] = res.exec_time_ns
    return out


# revision 33
# speedup vs baseline: 1.1809x; 1.1809x over previous
"""Trainium2 Bass kernel for multi-head attention (b=2, n=2048, d=512, h=8).

Sharding: batch*heads across 8 cores — each core computes full-sequence
attention for 2 heads of one batch element, plus its partial contribution
to the output projection. Host sums the 4 per-core partials per batch.

Per-core dataflow (fp16 operands, fp32 PSUM accumulation):
  A) Q.T, K.T = W.T-chunked @ x.T  (dh2=128 on partitions);
     V.T likewise, PE-transposed to V (keys on partitions) + fused ones col
  B) flash-style attention per 512-query tile, software-pipelined one tile
     deep: while ScalarE streams exp for tile t (both heads packed in one
     ACTIVATE; scale folded in), the PE runs PV+rowsum for tile t-1
     ([V_h|1].T @ P.T, accumulated in PSUM), normalization and the output
     projection for t-1 trail behind. Head0 runs on PE rows/cols 0-63 and
     head1 on 64-127 wherever both appear (row-tiled concurrent matmuls).
  C) partial out = O_norm.T.T @ Wo-cols (both heads as a concurrent
     K-split pair), DVE-copied to SBUF, DMA out fp16.
"""

import numpy as np

import concourse.bass as bass
import concourse.mybir as mybir
import concourse.tile as tile
from concourse import bacc
from concourse.bass_utils import run_bass_kernel_spmd
from concourse.masks import make_identity
from contextlib import ExitStack

P = 128          # partitions / key-chunk size
N = 2048         # sequence length
D = 512          # model dim
DH2 = 128        # head_dim * 2 local heads
QT = 512         # queries per tile
NQT = N // QT    # 4 query tiles
KC = N // P      # 16 key chunks
SCALE = D ** -0.5
F32 = mybir.dt.float32
F16 = mybir.dt.float16
EXP = mybir.ActivationFunctionType.Exp

_CACHED = {}


def build_nc():
    nc = bacc.Bacc("TRN2", target_bir_lowering=False, debug=False, num_devices=8)

    xt_d = nc.dram_tensor("xt", [P, 4, N], F16, kind="ExternalInput")
    wq_d = nc.dram_tensor("wq", [P, 4, DH2], F16, kind="ExternalInput")
    wk_d = nc.dram_tensor("wk", [P, 4, DH2], F16, kind="ExternalInput")
    wv_d = nc.dram_tensor("wv", [P, 4, DH2], F16, kind="ExternalInput")
    wo_d = nc.dram_tensor("wo", [64, 2, D], F16, kind="ExternalInput")
    out_d = nc.dram_tensor("out", [N, D], F16, kind="ExternalOutput")

    with tile.TileContext(nc) as tc, ExitStack() as ctx:
        const = ctx.enter_context(tc.tile_pool(name="const", bufs=1))
        xt_pool = ctx.enter_context(tc.tile_pool(name="xt", bufs=1))
        w_pool = ctx.enter_context(tc.tile_pool(name="w", bufs=1))
        qk_pool = ctx.enter_context(tc.tile_pool(name="qk", bufs=1))
        v_pool = ctx.enter_context(tc.tile_pool(name="v", bufs=1))
        pt_pool = ctx.enter_context(tc.tile_pool(name="pt", bufs=20))
        on_pool = ctx.enter_context(tc.tile_pool(name="on", bufs=2))
        nrm_pool = ctx.enter_context(tc.tile_pool(name="nrm", bufs=2))
        y_sb_pool = ctx.enter_context(tc.tile_pool(name="ysb", bufs=2))
        st_pool = ctx.enter_context(tc.tile_pool(name="st_ps", bufs=2, space="PSUM"))
        ot_pool = ctx.enter_context(tc.tile_pool(name="ot_ps", bufs=1, space="PSUM"))
        misc_ps = ctx.enter_context(tc.tile_pool(name="misc_ps", bufs=2, space="PSUM"))

        identity = const.tile([P, P], F16)
        make_identity(nc, identity[:])

        def fill_ones(out_ap):
            # DVE (in*0)+1 — writes exact fp16 ones
            nc.vector.tensor_scalar(
                out=out_ap, in0=identity[:, 0:out_ap.free_size()],
                scalar1=0.0, scalar2=1.0,
                op0=mybir.AluOpType.mult, op1=mybir.AluOpType.add,
            )

        # ---- input DMAs (fp16): x as 16 column-slice DMAs spread across
        # both HWDGE rings (sync + scalar) so many SDMA engines pull in
        # parallel; first slices match the first projection matmuls ----
        wk_sb = w_pool.tile([P, 4, DH2], F16, tag="wk")
        nc.sync.dma_start(wk_sb[:], wk_d.ap())
        wq_sb = w_pool.tile([P, 4, DH2], F16, tag="wq")
        nc.scalar.dma_start(wq_sb[:], wq_d.ap())
        wv_sb = w_pool.tile([P, 4, DH2], F16, tag="wv")
        nc.sync.dma_start(wv_sb[:], wv_d.ap())
        wo_sb = w_pool.tile([64, 2, D], F16, tag="wo")
        nc.scalar.dma_start(wo_sb[:], wo_d.ap())
        xt_sb = xt_pool.tile([P, 4, N], F16, tag="xt")
        rings = [nc.sync, nc.scalar]
        i = 0
        for tq4 in range(NQT):
            qs = slice(tq4 * QT, (tq4 + 1) * QT)
            for c in range(4):
                rings[i % 2].dma_start(xt_sb[:, c, qs], xt_d.ap()[:, c, qs])
                i += 1

        # ---- stage A: K and Q projections (V happens inside the t=0 slot) ----
        qT = qk_pool.tile([P, N], F16, tag="qT")
        kT = qk_pool.tile([P, N], F16, tag="kT")
        vT = qk_pool.tile([P, N], F16, tag="vT")
        v_sb = v_pool.tile([P, KC, 130], F16)

        def proj_tile(tgt, w_sb, tq4):
            ps = misc_ps.tile([P, QT], F32, tag="m", name=f"ps_{tgt.name}_{tq4}")
            for c in range(4):
                nc.tensor.matmul(
                    ps[:],
                    lhsT=w_sb[:, c, :],
                    rhs=xt_sb[:, c, tq4 * QT:(tq4 + 1) * QT],
                    start=(c == 0), stop=(c == 3),
                )
            nc.vector.tensor_copy(tgt[:, tq4 * QT:(tq4 + 1) * QT], ps[:])

        def v_path():
            for tq4 in range(NQT):
                ps = misc_ps.tile([P, QT], F32, tag="m", name=f"ps_v_{tq4}")
                for c in range(4):
                    nc.tensor.matmul(
                        ps[:],
                        lhsT=wv_sb[:, c, :],
                        rhs=xt_sb[:, c, tq4 * QT:(tq4 + 1) * QT],
                        start=(c == 0), stop=(c == 3),
                    )
                nc.vector.tensor_copy(vT[:, tq4 * QT:(tq4 + 1) * QT], ps[:])
            fill_ones(v_sb[:, :, 64])
            fill_ones(v_sb[:, :, 129])
            for c in range(KC):
                tp = misc_ps.tile([P, P], F16, tag="m", name=f"tp_{c}")
                nc.tensor.transpose(tp[:], vT[:, c * P:(c + 1) * P], identity[:])
                nc.vector.tensor_copy(v_sb[:, c, 0:64], tp[:, 0:64])
                nc.vector.tensor_copy(v_sb[:, c, 65:129], tp[:, 64:128])

        # ---- stage B/C: pipelined over query tiles ----
        pts = {}   # (t, c) -> pt tile
        ots = {}   # t -> (ot0, ot1)

        def st_act(t):
            st_act0(t, 0, KC)

        def st_act0(t, c0, c1):
            tq = slice(t * QT, (t + 1) * QT)
            for c in range(c0, c1):
                st = st_pool.tile([P, 2 * QT], F32, tag="st", name=f"st_{t}_{c}")
                for h in range(2):
                    hp = 64 * h
                    nc.tensor.matmul(
                        st[:, h * QT:(h + 1) * QT],
                        lhsT=kT[hp:hp + 64, c * P:(c + 1) * P],
                        rhs=qT[hp:hp + 64, tq],
                        start=True, stop=True,
                    )
                pt = pt_pool.tile([P, 2 * QT], F16, tag="pt", name=f"pt_{t}_{c}")
                nc.scalar.activation(pt[:], st[:], EXP, scale=SCALE)
                pts[(t, c)] = pt

        def pv_chunks(t, c0, c1):
            if c0 == 0:
                ots[t] = (ot_pool.tile([65, QT], F32, tag="ot0", name=f"ot0_{t}"),
                          ot_pool.tile([65, QT], F32, tag="ot1", name=f"ot1_{t}"))
            for c in range(c0, c1):
                pt = pts.pop((t, c))
                for h in range(2):
                    nc.tensor.matmul(
                        ots[t][h][:],
                        lhsT=v_sb[:, c, 65 * h:65 * h + 65],
                        rhs=pt[:, h * QT:(h + 1) * QT],
                        start=(c == 0), stop=(c == KC - 1),
                        skip_group_check=True,
                    )

        on_ts = {}

        def norm_head(t, h):
            # normalize one head (all operands partition-aligned at base 0)
            ot = ots[t][h]
            sums = nrm_pool.tile([1, QT], F32, tag="sums", name=f"sums_{t}_{h}")
            nc.vector.tensor_copy(sums[:], ot[64:65, :])
            rsum = nrm_pool.tile([1, QT], F32, tag="rsum", name=f"rsum_{t}_{h}")
            nc.vector.reciprocal_approx_fast(rsum[:], sums[:])
            rcb = nrm_pool.tile([64, QT], F32, tag="rcb", name=f"rcb_{t}_{h}")
            nc.gpsimd.partition_broadcast(rcb[:], rsum[:], channels=64)
            on_h = on_pool.tile([64, QT], F16, tag=f"on{h}", name=f"on_{t}_{h}")
            nc.vector.tensor_mul(on_h[:], ot[0:64, :], rcb[:])
            on_ts.setdefault(t, []).append(on_h)

        def out_proj(t):
            on_t = on_ts.pop(t)
            del ots[t]
            for qc in range(4):
                yps = misc_ps.tile([P, D], F32, tag="m", name=f"y_{t}_{qc}")
                for h in range(2):
                    nc.tensor.matmul(
                        yps[:],
                        lhsT=on_t[h][:, qc * P:(qc + 1) * P],
                        rhs=wo_sb[:, h, :],
                        start=(h == 0), stop=(h == 1),
                    )
                ysb = y_sb_pool.tile([P, D], F16, tag="ysb", name=f"ysb_{t}_{qc}")
                nc.vector.tensor_copy(ysb[:], yps[:])
                nc.sync.dma_start(
                    out_d.ap()[(t * 4 + qc) * P:(t * 4 + qc + 1) * P, :], ysb[:])

        # t=0: interleave K-projections with the first ST/ACT groups so the
        # exp stream starts as soon as kT-tile0 + qT-tile0 exist
        proj_tile(kT, wk_sb, 0)
        proj_tile(qT, wq_sb, 0)
        for tq4 in range(NQT):
            if tq4 > 0:
                proj_tile(kT, wk_sb, tq4)
            st_act0(0, tq4 * 4, tq4 * 4 + 4)
            if tq4 == 1:
                for q4 in range(1, NQT):
                    proj_tile(qT, wq_sb, q4)
            if tq4 == 2:
                v_path()

        # PV for tile t-1 runs while the ST/ACT stream of tile t proceeds
        for t in range(1, NQT + 1):
            if t < NQT:
                st_act(t)
            pv_chunks(t - 1, 0, KC)
            norm_head(t - 1, 0)
            norm_head(t - 1, 1)
            out_proj(t - 1)

    nc.compile()
    return nc


def make_in_maps(x, Wq, Wk, Wv, Wo):
    """Shard full inputs into the 8 per-core input dicts (host-side fp16)."""
    in_maps = []
    for core in range(8):
        b, p = divmod(core, 4)
        r = slice(p * DH2, (p + 1) * DH2)
        # xt[p, c, n] = x[b, n, c*128 + p]
        xt = x[b].T.reshape(4, P, N).transpose(1, 0, 2)
        wq = Wq[r, :].T.reshape(4, P, DH2).transpose(1, 0, 2)
        wk = Wk[r, :].T.reshape(4, P, DH2).transpose(1, 0, 2)
        wv = Wv[r, :].T.reshape(4, P, DH2).transpose(1, 0, 2)
        wo = Wo[:, r].T.reshape(2, 64, D).transpose(1, 0, 2)
        in_maps.append({
            "xt": np.ascontiguousarray(xt, dtype=np.float16),
            "wq": np.ascontiguousarray(wq, dtype=np.float16),
            "wk": np.ascontiguousarray(wk, dtype=np.float16),
            "wv": np.ascontiguousarray(wv, dtype=np.float16),
            "wo": np.ascontiguousarray(wo, dtype=np.float16),
        })
    return in_maps


def kernel(x, mask, Wq, Wk, Wv, Wo, bo, _trace=False):
    x = np.asarray(x, dtype=np.float32)
    Wq = np.asarray(Wq, dtype=np.float32)
    Wk = np.asarray(Wk, dtype=np.float32)
    Wv = np.asarray(Wv, dtype=np.float32)
    Wo = np.asarray(Wo, dtype=np.float32)
    bo = np.asarray(bo, dtype=np.float32)
    # mask is additive and all-zeros per the problem spec -> identity, ignored

    if "nc" not in _CACHED:
        _CACHED["nc"] = build_nc()
    nc = _CACHED["nc"]

    in_maps = make_in_maps(x, Wq, Wk, Wv, Wo)
    res = run_bass_kernel_spmd(nc, in_maps, core_ids=list(range(8)), trace=_trace)
    parts = [res.results[c]["out"].astype(np.float32) for c in range(8)]
    out = np.empty((2, N, D), dtype=np.float32)
    for b in range(2):
        out[b] = parts[4 * b] + parts[4 * b + 1] + parts[4 * b + 2] + parts[4 * b + 3]
    out += bo[None, None, :]
    _CACHED["last_exec_time_ns"] = res.exec_time_ns
    return out
